# revision 1
# baseline (speedup 1.0000x reference)
import sys

sys.path.insert(0, "/opt/trn_rl_repo")

import numpy as np
import ml_dtypes

import concourse.mybir as mybir
from concourse import bass, tile
from concourse import tile_sem_assignment as _tsa
from concourse.bass_utils import run_bass_kernel_spmd
from concourse.vector_clock import ScopedClock, VectorClock

_orig_drain_and_barrier = tile.TileContext._drain_and_barrier


def _split_drain_and_barrier(self, tick_clock, wait_clock):
    # The final Drain waits on every active semaphore at once; with 8 HWDGE
    # lanes + SWDGE + 3 engines that exceeds the CTRL instruction's sync
    # wait slots. Emit one 1-wait drain per proc instead (same semantics:
    # SP executes them in order, so all sems reach their targets before the
    # barrier), then replicate the original barrier/cleanup sequence.
    gc = tick_clock.global_clock
    n = _tsa.N_PROCS
    for p in range(n):
        if gc[p] > 0:
            partial = VectorClock([gc[q] if q == p else 0 for q in range(n)])
            d = self.nc.sync.drain()
            wait_clock.add_sem_waits(d.ins, ScopedClock({None: partial}))
    self.nc.all_engine_barrier()
    popped = self.nc._tile_sem_poison_stack.pop()
    assert popped is self._sem_poison
    self.nc.clear_and_free_semaphores(list(self.sems.allocated().values()))
    self.nc.all_engine_barrier()


tile.TileContext._drain_and_barrier = _split_drain_and_barrier

B = 1024        # batch rows of address
N = 65536       # mem rows (sharded)
M = 128         # mem cols
NCORES = 8
NS = N // NCORES          # 8192 rows per core
NCHUNKS = NS // 128       # 64 chunks of 128 mem-rows
MCHUNKS = NS // 256       # 32 mega-chunks of 256 mem-rows (DoubleRow)
BCHUNKS = B // 128        # 8 chunks of 128 batch-rows
NSTAGES = 8               # DMA pipeline stages (8 chunks each)

FP8 = mybir.dt.float8e4
BF16 = mybir.dt.bfloat16
F32 = mybir.dt.float32
DR = mybir.MatmulPerfMode.DoubleRow
ADD = mybir.AluOpType.add
MULT = mybir.AluOpType.mult

_compiled = {}


NG = 16  # a DMA groups; each covers 4 mem-chunks (k) = 2 mega-chunks (ch)


def _build_nc():
    nc = bass.Bass(target_bir_lowering=False)

    # a:  [p=b%128, j(n-slice of 1024), ub(u-block), bc, u]  A shard for GEMM1
    a = nc.dram_tensor("a", [128, NSTAGES, 8, BCHUNKS, 128], FP8, kind="ExternalInput")
    # at: [p=n%128 within 256-chunk, ch, sub, b]  A^T shard for GEMM2 (partition=n)
    at = nc.dram_tensor("at", [128, MCHUNKS, 2, B], FP8, kind="ExternalInput")
    # c:  [p=n%128, k, m]  0.5*content shard (partition=n)
    c = nc.dram_tensor("c", [128, NCHUNKS, M], FP8, kind="ExternalInput")
    # ed: [p=b%128, bc, 2M]  [-erase | 0.5*add] fp8 (hi only)
    ed = nc.dram_tensor("ed", [128, BCHUNKS, 2 * M], FP8, kind="ExternalInput")
    # rt: [m, b] partial (read/2)^T bf16
    rt = nc.dram_tensor("rt", [M, B], BF16, kind="ExternalOutput")

    with tile.TileContext(nc) as tc:
        with (
            tc.tile_pool(name="abuf", bufs=1) as a_pool,
            tc.tile_pool(name="atbuf", bufs=1) as at_pool,
            tc.tile_pool(name="cbuf", bufs=1) as c_pool,
            tc.tile_pool(name="edbuf", bufs=1) as ed_pool,
            tc.tile_pool(name="tmpbuf", bufs=8) as tmp_pool,
            tc.tile_pool(name="cpbuf", bufs=6) as cp_pool,
            tc.tile_pool(name="rtbuf", bufs=2) as rt_pool,
            tc.tile_pool(name="pw", bufs=2, space="PSUM") as pw_pool,
            tc.tile_pool(name="pr", bufs=1, space="PSUM") as pr_pool,
        ):
            a_t = a_pool.tile([128, NSTAGES, 8, BCHUNKS, 128], FP8)
            at_t = at_pool.tile([128, MCHUNKS, 2, B], FP8)
            c_t = c_pool.tile([128, NCHUNKS, M], FP8)
            ed_t = ed_pool.tile([128, BCHUNKS, 2 * M], FP8)

            # Transfers issued from different queues run concurrently in the
            # model (the engine SEQ is the serial resource, ~1.58us per
            # 512KB DMA), so spread the input DMAs across all three DMA
            # queues (SP + Act HWDGE, Pool SWDGE) round-robin in global
            # consumption order: DVE eats one n-chunk per ~310ns and needs
            # the matching a-group and at-pair at the same cadence, with a
            # c quarter every 16 chunks. Preload DMAs write each SBUF dest
            # exactly once, so their only wait is the DGE lane-credit wait.
            pieces = []

            def a_group(g):
                j, ub0 = g // 2, (g % 2) * 4
                pieces.append((a_t[:, j, ub0 : ub0 + 4], a[:, j, ub0 : ub0 + 4]))

            def at_pair(p):
                pieces.append((at_t[:, 2 * p : 2 * p + 2], at[:, 2 * p : 2 * p + 2]))

            def c_quarter(qi):
                pieces.append(
                    (c_t[:, 16 * qi : 16 * qi + 16, :], c[:, 16 * qi : 16 * qi + 16, :])
                )

            pieces.append((ed_t[:], ed[:]))
            a_group(0)
            c_quarter(0)
            a_group(1)
            a_group(2)
            at_pair(0)
            a_group(3)
            at_pair(1)
            c_quarter(1)
            for g in range(4, 16):
                a_group(g)
                if g == 7:
                    c_quarter(2)
                if g == 11:
                    c_quarter(3)
                at_pair(g - 2)
            at_pair(14)
            pieces.append((at_t[:, 30:31], at[:, 30:31]))
            pieces.append((at_t[:, 31:32, :, 0:512], at[:, 31:32, :, 0:512]))
            pieces.append((at_t[:, 31:32, :, 512:1024], at[:, 31:32, :, 512:1024]))

            # Pool's first op: memset the dummy-matmul source so the PE
            # p-state warm-up (below) can start immediately.
            wsrcm = tmp_pool.tile([128, 512], FP8)
            nc.gpsimd.memset(wsrcm[:], 0.0)

            queues = [nc.sync, nc.scalar, nc.gpsimd]
            for i, (dst, srcp) in enumerate(pieces):
                queues[i % 3].dma_start(out=dst, in_=srcp)

            # Ramp the PE p-state before the first real G1: the model runs
            # the Tensor engine at 1.2GHz until it has been continuously
            # busy for 3us. Dummy matmuls into psum_r0 (discarded by the
            # first real G2's start=True) span the window until a0 lands.
            psum_r0 = pr_pool.tile([128, 512], F32)
            psum_r1 = pr_pool.tile([128, 512], F32)
            psum_r = [psum_r0, psum_r1]
            for _ in range(8):
                nc.tensor.matmul(
                    psum_r0[0:1, :], wsrcm[:, 0:1], wsrcm[:], start=True, stop=True
                )

            land = tmp_pool.tile([128, 1], F32)
            # Wake the Activation engine early: its first instruction carries
            # a ~1.3us act-table load in the model; pay it off the critical
            # path so the tail copy runs at steady-state rate.
            warm = tmp_pool.tile([128, 1], F32)
            nc.scalar.copy(warm[:], wsrcm[:, 0:1])

            def emit_g2(ch, cp):
                for jj in range(2):
                    nc.tensor.matmul(
                        psum_r[jj][:],
                        cp[:],
                        at_t[:, ch, :, jj * 512 : (jj + 1) * 512],
                        start=(ch == 0),
                        stop=(ch == MCHUNKS - 1),
                        perf_mode=DR,
                    )

            # Process 6 n-chunks per iteration (last group 4): the G1s land
            # in one 3-bank psum tile, and the update runs as ONE fused
            # STT/TT pair over [128, w, M] (strided psum AP), amortizing the
            # per-instruction DVE overhead across 6 chunks.
            for k0, w in (
                [(0, 4)]
                + [(kk, 6) for kk in range(4, 58, 6)]
                + [(58, 4), (62, 2)]
            ):
                # DVE absorbs the c DMA wait for this group's last chunk so
                # STT(k0) keeps only its PSUM-read wait (later c-lane waits
                # dedup against the DVE clock).
                nc.vector.tensor_copy(land[:], c_t[:, k0 + w - 1, 0:1])

                cp = cp_pool.tile([128, 6, M], FP8)
                psum_w = pw_pool.tile([128, 6, 2 * M], F32)
                for dk in range(w):
                    k = k0 + dk
                    j, ub = k // 8, k % 8
                    for q in range(4):
                        nc.tensor.matmul(
                            psum_w[:, dk, :],
                            a_t[:, j, ub, 2 * q : 2 * q + 2, :],
                            ed_t[:, 2 * q : 2 * q + 2, :],
                            start=(q == 0),
                            stop=(q == 3),
                            perf_mode=DR,
                        )

                # psum_w[dk] = [-We | Wa/2];  C'/2 = (1 - We)*(C/2) + Wa/2
                tmp2 = tmp_pool.tile([128, 6, M], F32)
                nc.vector.scalar_tensor_tensor(
                    tmp2[:, 0:w, :],
                    psum_w[:, 0:w, 0:M],
                    1.0,
                    c_t[:, k0 : k0 + w, :],
                    ADD,
                    MULT,
                )
                nc.vector.tensor_add(
                    cp[:, 0:w, :], tmp2[:, 0:w, :], psum_w[:, 0:w, M : 2 * M]
                )

                for dch in range(w // 2):
                    emit_g2(k0 // 2 + dch, cp[:, 2 * dch : 2 * dch + 2, :])

            # Tail: psum_r0 copies on DVE (free right after its last TT)
            # and stores via Pool; psum_r1 copies on Act and stores via SP.
            # Each store rides a DIFFERENT queue than its copying engine so
            # its RAW wait is cross-engine and must never be dropped (a
            # same-queue store's DMA read can race its own copy's engine
            # pass on hardware even though queue dispatch is in-order).
            rt_t0 = rt_pool.tile([128, 512], BF16)
            rt_t1 = rt_pool.tile([128, 512], BF16)
            nc.vector.tensor_copy(rt_t0[:], psum_r0[:])
            s0 = nc.gpsimd.dma_start(out=rt[:, 0:512], in_=rt_t0[:])
            nc.scalar.copy(rt_t1[:], psum_r1[:])
            s1 = nc.sync.dma_start(out=rt[:, 512:1024], in_=rt_t1[:])
            store_names = {s0.ins.name, s1.ins.name}

    # The scheduler can hoist    # The scheduler can hoist a G1 start-Matmult ahead of the G2 Ldweights
    # whose DVE wait would dedup-cover its bank-WAR wait, leaving it with
    # two waits (PE self-wait + DVE) — one over the HW wait-slot limit.
    # The same-engine self-wait is always satisfied by in-order queue
    # completion, so drop it.
    # The rt stores' RAW wait (on the tail Act copy) transitively follows
    # every input DMA completing, so a DMA-lane credit wait on them is
    # always already satisfied — drop it to stay within the 1-wait limit.
    for inst in nc.inst_map.values():
        si = inst.sync_info
        if si and si.on_wait and len(si.on_wait) > 1:
            eng = str(inst.engine).split(".")[-1]
            is_dma = "DMA" in type(inst).__name__ or "Dma" in type(inst).__name__
            if is_dma:
                # Never drop engine waits on a DMA (its transfer is async
                # from queue dispatch); only the rt stores' lane-credit
                # waits are provably dominated by their cross-engine RAW
                # wait (the copy follows every input DMA completing).
                assert inst.name in store_names, (
                    inst.name,
                    [w.ant_name for w in si.on_wait],
                )
                kept = [w for w in si.on_wait if not w.ant_name.startswith("DMA")]
            else:
                kept = [w for w in si.on_wait if not w.ant_name.startswith(eng + "_")]
            assert len(kept) == 1, (inst.name, [w.ant_name for w in si.on_wait])
            si.on_wait = kept

    return nc


def _prep_inputs(address, erase, add, content):
    f8 = ml_dtypes.float8_e4m3
    a_f8 = address.astype(f8)                                 # [1024, 65536]
    ed = np.concatenate([-erase, 0.5 * add], axis=1)          # [1024, 256] f32
    ed_r = np.ascontiguousarray(
        ed.astype(f8).reshape(BCHUNKS, 128, 2 * M).transpose(1, 0, 2)
    )                                                         # [128, 8, 256]
    c_f8 = (0.5 * content).astype(f8)                         # [65536, 128]

    in_maps = []
    for ci in range(NCORES):
        a_c = a_f8[:, ci * NS : (ci + 1) * NS]                # [1024, 8192]
        # a_r[p, j, ub, bc, u] = A[bc*128+p, j*1024+ub*128+u]
        a_r = np.ascontiguousarray(
            a_c.reshape(BCHUNKS, 128, NSTAGES, 8, 128).transpose(1, 2, 3, 0, 4)
        )                                                     # [128, 8, 8, 8, 128]
        # at_r[p, ch, s, b] = A[b, ch*256 + s*128 + p]
        at_r = np.ascontiguousarray(
            a_c.T.reshape(MCHUNKS, 2, 128, B).transpose(2, 0, 1, 3)
        )                                                     # [128, 32, 2, 1024]
        c_c = c_f8[ci * NS : (ci + 1) * NS, :]
        c_r = np.ascontiguousarray(
            c_c.reshape(NCHUNKS, 128, M).transpose(1, 0, 2)
        )                                                     # [128, 64, 128]
        in_maps.append({"a": a_r, "at": at_r, "c": c_r, "ed": ed_r})
    return in_maps


def kernel(address, erase, add, content, _trace=False, _result_box=None):
    if "nc" not in _compiled:
        _compiled["nc"] = _build_nc()
    nc = _compiled["nc"]

    in_maps = _prep_inputs(address, erase, add, content)
    res = run_bass_kernel_spmd(
        nc, in_maps, core_ids=list(range(NCORES)), trace=_trace
    )
    if _result_box is not None:
        _result_box.append(res)

    acc = np.zeros((M, B), dtype=np.float32)
    for r in res.results:
        acc += np.asarray(r["rt"], dtype=np.float32)
    return np.ascontiguousarray((2.0 * acc).T)



# revision 51
# speedup vs baseline: 1.0213x; 1.0213x over previous
import sys

sys.path.insert(0, "/opt/trn_rl_repo")

import numpy as np
import ml_dtypes

import concourse.mybir as mybir
from concourse import bass, tile
from concourse import tile_sem_assignment as _tsa
from concourse.bass_utils import run_bass_kernel_spmd
from concourse.vector_clock import ScopedClock, VectorClock

_orig_drain_and_barrier = tile.TileContext._drain_and_barrier


def _split_drain_and_barrier(self, tick_clock, wait_clock):
    # The final Drain waits on every active semaphore at once; with 8 HWDGE
    # lanes + SWDGE + 3 engines that exceeds the CTRL instruction's sync
    # wait slots. Emit one 1-wait drain per proc instead (same semantics:
    # SP executes them in order, so all sems reach their targets before the
    # barrier), then replicate the original barrier/cleanup sequence.
    gc = tick_clock.global_clock
    n = _tsa.N_PROCS
    for p in range(n):
        if gc[p] > 0:
            partial = VectorClock([gc[q] if q == p else 0 for q in range(n)])
            d = self.nc.sync.drain()
            wait_clock.add_sem_waits(d.ins, ScopedClock({None: partial}))
    self.nc.all_engine_barrier()
    popped = self.nc._tile_sem_poison_stack.pop()
    assert popped is self._sem_poison
    self.nc.clear_and_free_semaphores(list(self.sems.allocated().values()))
    self.nc.all_engine_barrier()


tile.TileContext._drain_and_barrier = _split_drain_and_barrier

B = 1024        # batch rows of address
N = 65536       # mem rows (sharded)
M = 128         # mem cols
NCORES = 8
NS = N // NCORES          # 8192 rows per core
NCHUNKS = NS // 128       # 64 chunks of 128 mem-rows
MCHUNKS = NS // 256       # 32 mega-chunks of 256 mem-rows (DoubleRow)
BCHUNKS = B // 128        # 8 chunks of 128 batch-rows
NSTAGES = 8               # a-tensor j blocks (8 chunks each)

FP8 = mybir.dt.float8e4
BF16 = mybir.dt.bfloat16
F32 = mybir.dt.float32
DR = mybir.MatmulPerfMode.DoubleRow
ADD = mybir.AluOpType.add
MULT = mybir.AluOpType.mult

_compiled = {}

# G1 chunk groups: (first chunk, width, pool_pairs). The update
# (C'/2 = (1-We)(C/2) + Wa/2) is split by whole ch-pairs between DVE and
# Pool: Pool takes the last two chunks of most groups INTO ITS OWN PSUM
# TILES (psum access serializes per tile, so sharing one tile would chain
# Pool behind DVE and gate the G1 WAR on Pool's DMA-busy queue). Every G2
# weight load depends on exactly one engine's cp write (single-wait).
# Processing order: the (58,4) group runs FIRST so chs 29/30's cp is ready
# early; chunks 62,63 (ch31) run last and only their tiny chain is in the
# tail. psum_r0 accumulates [29, 0..28, 30(stop)] - the stop fires on
# long-ready data right after ch28, letting Act evacuate it mid-stream.
# psum_r1 accumulates [29, 30, 0..28, 31(stop)]. ch31's jj0 lands in psum_x
# (carved from the last group's pw slot) and merges via one DVE tensor_add.
GROUPS = (
    [(58, 4, 0)]
    + [(kk, 6, 0) for kk in range(0, 54, 6)]
    + [(54, 4, 0), (62, 2, 0)]
)

X_CH = 31


def _build_nc():
    nc = bass.Bass(target_bir_lowering=False)

    # a:  [p=b%128, j(n-slice of 1024), ub(u-block), bc, u]  A shard for GEMM1
    a = nc.dram_tensor("a", [128, NSTAGES, 8, BCHUNKS, 128], FP8, kind="ExternalInput")
    # at: [p=n%128 within 256-chunk, ch, sub, b]  A^T shard for GEMM2 (partition=n)
    at = nc.dram_tensor("at", [128, MCHUNKS, 2, B], FP8, kind="ExternalInput")
    # c:  [p=n%128, k, m]  0.5*content shard (partition=n)
    c = nc.dram_tensor("c", [128, NCHUNKS, M], FP8, kind="ExternalInput")
    # ed: [p=b%128, bc, 2M]  [-erase | 0.5*add] fp8 (hi only)
    ed = nc.dram_tensor("ed", [128, BCHUNKS, 2 * M], FP8, kind="ExternalInput")
    # rt: [m, b] partial (read/2)^T bf16
    rt = nc.dram_tensor("rt", [M, B], BF16, kind="ExternalOutput")

    store_names = set()

    with tile.TileContext(nc) as tc:
        with (
            tc.tile_pool(name="abuf", bufs=1) as a_pool,
            tc.tile_pool(name="atbuf", bufs=1) as at_pool,
            tc.tile_pool(name="cbuf", bufs=1) as c_pool,
            tc.tile_pool(name="edbuf", bufs=1) as ed_pool,
            tc.tile_pool(name="tmpbuf", bufs=6) as tmp_pool,
            tc.tile_pool(name="tmppbuf", bufs=6) as tmpp_pool,
            tc.tile_pool(name="landbuf", bufs=1) as land_pool,
            tc.tile_pool(name="landpbuf", bufs=1) as landp_pool,
            tc.tile_pool(name="wsrcbuf", bufs=1) as wsrc_pool,
            tc.tile_pool(name="cpbuf", bufs=12) as cp_pool,
            tc.tile_pool(name="rtbuf", bufs=2) as rt_pool,
            tc.tile_pool(name="warmbuf", bufs=1) as warm_pool,
            tc.tile_pool(name="pw", bufs=2, space="PSUM") as pw_pool,
            tc.tile_pool(name="pwp", bufs=2, space="PSUM") as pwp_pool,
            tc.tile_pool(name="pr", bufs=1, space="PSUM") as pr_pool,
        ):
            a_t = a_pool.tile([128, NSTAGES, 8, BCHUNKS, 128], FP8)
            at_t = at_pool.tile([128, MCHUNKS, 2, B], FP8)
            c_t = c_pool.tile([128, NCHUNKS, M], FP8)
            ed_t = ed_pool.tile([128, BCHUNKS, 2 * M], FP8)

            # Pool's first op: memset a small dummy-matmul source so PE
            # p-state warm-up can start almost immediately.
            wsrcm = wsrc_pool.tile([128, 64], FP8)
            nc.gpsimd.memset(wsrcm[:], 0.0)

            # DMA pieces, explicit queue assignment (SP/Act HWDGE, Pool
            # SWDGE), each queue's list in consumption order. First waves are
            # small so the first G1 chunks can start ~2.3us at MID clock.
            # Pool carries fewer bytes because it also runs the [MD:M] column
            # slice of every group's update.
            pieces = []
            SP, AC, PL = "sp", "ac", "pl"

            def q(which, dst, srcp):
                pieces.append((which, dst, srcp))

            # Explicit deadline-driven plan. The early phase is G1-only
            # (G2s lag the first update), demanding `a` at ~600GB/s, so `a`
            # pieces are front-loaded on all three queues in small pieces;
            # at/c slot in as their consumers approach; late at-chs park at
            # the ends. Per-queue order == consumption order.
            q(SP, ed_t[:, 0:4], ed[:, 0:4])
            q(AC, ed_t[:, 4:8], ed[:, 4:8])
            q(PL, a_t[:, 0, 0:1], a[:, 0, 0:1])
            q(SP, a_t[:, 0, 1:2], a[:, 0, 1:2])
            q(AC, a_t[:, 0, 2:3], a[:, 0, 2:3])
            q(PL, c_t[:, 0:8, :], c[:, 0:8, :])
            q(SP, a_t[:, 0, 3:4], a[:, 0, 3:4])
            q(AC, a_t[:, 0, 5:6], a[:, 0, 5:6])
            q(PL, a_t[:, 0, 4:5], a[:, 0, 4:5])
            q(SP, a_t[:, 0, 6:7], a[:, 0, 6:7])
            q(AC, a_t[:, 0, 7:8], a[:, 0, 7:8])
            q(SP, a_t[:, 1, 0:4], a[:, 1, 0:4])
            q(AC, at_t[:, 0:2], at[:, 0:2])
            q(PL, at_t[:, 2:5], at[:, 2:5])
            q(SP, a_t[:, 2, 0:4], a[:, 2, 0:4])
            q(AC, a_t[:, 1, 4:8], a[:, 1, 4:8])
            q(PL, c_t[:, 8:26, :], c[:, 8:26, :])
            q(SP, c_t[:, 26:40, :], c[:, 26:40, :])
            q(AC, at_t[:, 5:7], at[:, 5:7])
            q(PL, a_t[:, 2, 4:8], a[:, 2, 4:8])
            q(SP, a_t[:, 3, 0:4], a[:, 3, 0:4])
            q(AC, a_t[:, 3, 4:8], a[:, 3, 4:8])
            q(PL, at_t[:, 7:8], at[:, 7:8])
            q(SP, a_t[:, 4, 0:4], a[:, 4, 0:4])
            q(AC, at_t[:, 8:10], at[:, 8:10])
            q(PL, a_t[:, 5, 4:8], a[:, 5, 4:8])
            q(SP, at_t[:, 14:16], at[:, 14:16])
            q(AC, a_t[:, 4, 4:8], a[:, 4, 4:8])
            q(PL, at_t[:, 10:12], at[:, 10:12])
            q(SP, a_t[:, 5, 0:4], a[:, 5, 0:4])
            q(AC, at_t[:, 16:18], at[:, 16:18])
            q(PL, at_t[:, 12:14], at[:, 12:14])
            q(SP, a_t[:, 6, 0:4], a[:, 6, 0:4])
            q(AC, a_t[:, 6, 4:8], a[:, 6, 4:8])
            q(PL, c_t[:, 40:56, :], c[:, 40:56, :])
            q(SP, at_t[:, 20:22], at[:, 20:22])
            q(AC, at_t[:, 22:24], at[:, 22:24])
            q(PL, at_t[:, 18:20], at[:, 18:20])
            q(SP, a_t[:, 7, 0:4], a[:, 7, 0:4])
            q(AC, a_t[:, 7, 4:8], a[:, 7, 4:8])
            q(PL, at_t[:, 24:26], at[:, 24:26])
            q(SP, at_t[:, 26:28], at[:, 26:28])
            q(AC, at_t[:, 28:30], at[:, 28:30])
            q(PL, c_t[:, 56:64, :], c[:, 56:64, :])
            q(SP, at_t[:, 30:32], at[:, 30:32])

            queues = {SP: nc.sync, AC: nc.scalar, PL: nc.gpsimd}
            for which, dst, srcp in pieces:
                queues[which].dma_start(out=dst, in_=srcp)

            # Ramp the PE p-state: the model runs the Tensor engine at 1.2GHz
            # until it has been continuously busy for 3us. Small dummy matmuls
            # into psum_r0 (discarded by the first real G2's start=True) keep
            # PE busy until the first a/ed pieces land (~2.3us); the first
            # real chunks then run at MID clock until the ramp completes.
            psum_r0 = pr_pool.tile([128, 2, 256], F32)
            psum_r1 = pr_pool.tile([128, 2, 256], F32)
            psum_r = [psum_r0, psum_r1]
            for _ in range(42):
                nc.tensor.matmul(
                    psum_r0[0:1, 0, 0:64], wsrcm[:, 0:1], wsrcm[:], start=True, stop=True
                )

            # Per-partition scalar tiles for the STTs. Refreshing them with
            # ops that READ the c-DMA boundary chunks (and the recycled cp
            # slot) forces those waits onto the cheap refresher via a real
            # data edge, so the big ops keep a single cross-engine wait.
            oned = land_pool.tile([128, 1], F32)
            zerod = land_pool.tile([128, 1], F32, name="zerod")
            onep = landp_pool.tile([128, 1], F32)

            g2_first = [True]

            def emit_g2(ch, cp, out=None, start=None, stop=False):
                for jj in range(2):
                    dst = psum_r[jj][:] if out is None else out[:, jj, :]
                    nc.tensor.matmul(
                        dst,
                        cp[:],
                        at_t[:, ch, :, jj * 512 : (jj + 1) * 512],
                        start=g2_first[0] if start is None else start,
                        stop=stop,
                        perf_mode=DR,
                    )
                    if start is None:
                        g2_first[0] = False

            def g1_mms(k0, nk, psum_w, d0):
                for dk in range(nk):
                    k = k0 + dk
                    j, ub = k // 8, k % 8
                    for q4 in range(4):
                        nc.tensor.matmul(
                            psum_w[:, d0 + dk, :],
                            a_t[:, j, ub, 2 * q4 : 2 * q4 + 2, :],
                            ed_t[:, 2 * q4 : 2 * q4 + 2, :],
                            start=(q4 == 0),
                            stop=(q4 == 3),
                            perf_mode=DR,
                        )

            def emit_update(k0, w, cp, pool_pairs=0, psum_w=None, nxt=None,
                            pool_all=False):
                # psum_w[dk] = [-We | Wa/2];  C'/2 = (1 - We)*(C/2) + Wa/2
                # DVE handles chunks [0:wd] in its psum tile, Pool the last
                # pool_pairs*2 in a separate tile; the land copies let each
                # engine absorb the c-DMA wait before its STT so every
                # instruction keeps a single cross-engine wait.
                if pool_all:
                    # the final tiny group updates on Pool (its DMA queue has
                    # long drained by now), in parallel with DVE's tail
                    g1_mms(k0, w, psum_w, 0)
                    # onep := 1.0 reading the c piece, so the STT's c-wait is
                    # forced through a real data edge (scalar operand)
                    nc.gpsimd.tensor_scalar(
                        onep[:], c_t[:, k0 + w - 1, 1:2], 0.0, 1.0, MULT, ADD
                    )
                    tmp2p = tmpp_pool.tile([128, w, M], F32)
                    nc.gpsimd.scalar_tensor_tensor(
                        tmp2p[:, 0:w, :],
                        psum_w[:, 0:w, 0:M],
                        onep[:],
                        c_t[:, k0 : k0 + w, :],
                        ADD,
                        MULT,
                    )
                    nc.gpsimd.tensor_add(
                        cp[:, 0:w, :], tmp2p[:, 0:w, :], psum_w[:, 0:w, M : 2 * M]
                    )
                    return
                wd = w - 2 * pool_pairs
                if psum_w is None:
                    psum_w = pw_pool.tile([128, 6, 2 * M], F32)
                g1_mms(k0, wd, psum_w, 0)
                if pool_pairs:
                    psum_wp = pwp_pool.tile([128, 2, 2 * M], F32)
                    g1_mms(k0 + wd, 2, psum_wp, 0)
                # oned/onep are always 1.0; the refreshers exist to absorb
                # the c-DMA waits via a real data edge, and are hoisted one
                # group ahead (emitted mid-update) so their sem waits are
                # long-satisfied and add no latency to the DVE/Pool chains.
                tmp2 = tmp_pool.tile([128, wd, M], F32)
                nc.vector.scalar_tensor_tensor(
                    tmp2[:, 0:wd, :],
                    psum_w[:, 0:wd, 0:M],
                    oned[:],
                    c_t[:, k0 : k0 + wd, :],
                    ADD,
                    MULT,
                )
                for cell in (nxt or ()):
                    # span-entry refresher: pulls the new c-piece's DMA sem
                    # into DVE's wait clock one group early
                    nc.vector.tensor_scalar(
                        oned[:], c_t[:, cell, 0:1], 0.0, 1.0, MULT, ADD
                    )
                nc.vector.tensor_add(
                    cp[:, 0:wd, :], tmp2[:, 0:wd, :], psum_w[:, 0:wd, M : 2 * M]
                )
                if pool_pairs:
                    tmp2p = tmpp_pool.tile([128, 2, M], F32)
                    nc.gpsimd.scalar_tensor_tensor(
                        tmp2p[:, 0:2, :],
                        psum_wp[:, 0:2, 0:M],
                        onep[:],
                        c_t[:, k0 + wd : k0 + w, :],
                        ADD,
                        MULT,
                    )
                    if nxt is not None and nxt[1] is not None:
                        nc.gpsimd.tensor_scalar(
                            onep[:], c_t[:, nxt[1], 1:2], 0.0, 1.0, MULT, ADD
                        )
                    nc.gpsimd.tensor_add(
                        cp[:, wd:w, :], tmp2p[:, 0:2, :], psum_wp[:, 0:2, M : 2 * M]
                    )

            # The last group's psum_w and psum_x share one pw-shaped slot:
            # 8 psum banks leave no room for a third concurrent pw tile.
            pwx = [None]
            last_gi = len(GROUPS) - 1
            cp30 = [None]
            cp_of_last = [None]
            C_SPANS = ((0, 8), (8, 26), (26, 40), (40, 56), (56, 64))
            spans_waited = {(56, 64)}  # covered by the oned seed
            # seed the scalar tile (the hoisted refreshers keep it at 1.0)
            nc.vector.tensor_scalar(
                oned[:], c_t[:, 0, 0:1], 0.0, 1.0, MULT, ADD
            )
            for gi, (k0, w, pp) in enumerate(GROUPS):
                cp = cp_pool.tile([128, 6, M], FP8)
                # c-span cells the NEXT group newly touches (hoisted refs)
                nxt = []
                if gi + 1 <= last_gi:
                    nk0, nw, npp = GROUPS[gi + 1]
                    for k in range(nk0, nk0 + nw - 2 * npp):
                        sp = next(s for s in C_SPANS if s[0] <= k < s[1])
                        if sp not in spans_waited:
                            spans_waited.add(sp)
                            nxt.append(min(sp[1] - 1, nk0 + nw - 2 * npp - 1))
                            break
                if gi == last_gi:
                    pwx[0] = pw_pool.tile([128, 6, 2 * M], F32, name="psum_w")
                    cp_of_last[0] = cp[:, 0:2, :]
                    emit_update(k0, w, cp, pool_pairs=pp,
                                psum_w=pwx[0][:, 0:w, :], nxt=nxt)
                else:
                    emit_update(k0, w, cp, pool_pairs=pp, nxt=nxt)

                def g2mm(jj, ch, cpg, start, stop, out=None):
                    dst = out if out is not None else psum_r[jj]
                    nc.tensor.matmul(
                        dst[:],
                        cpg[:],
                        at_t[:, ch, :, jj * 512 : (jj + 1) * 512],
                        start=start,
                        stop=stop,
                        perf_mode=DR,
                    )

                for dch in range(w // 2):
                    ch = k0 // 2 + dch
                    cpg = cp[:, 2 * dch : 2 * dch + 2, :]
                    if ch == X_CH:
                        continue  # emitted in the tail
                    if ch == 29:
                        # first accumulated into both banks
                        g2mm(0, ch, cpg, True, False)
                        g2mm(1, ch, cpg, True, False)
                    elif ch == 30:
                        # jj1 early; jj0 deferred to be psum_r0's stop
                        g2mm(1, ch, cpg, False, False)
                        cp30[0] = cpg
                    else:
                        g2mm(0, ch, cpg, False, False)
                        g2mm(1, ch, cpg, False, False)
                    if ch == 28:
                        # bank jj0 completes with long-ready ch30
                        g2mm(0, 30, cp30[0], False, True)

                if gi == 7:
                    # Warm the Activation engine's table load off the tail's
                    # critical path; the c_t read keeps it out of Act's busy
                    # early DMA window.
                    warm = warm_pool.tile([128, 1], F32)
                    nc.scalar.copy(warm[:], c_t[:, 57, 0:1])

                if gi == 10:
                    # psum_r0 stopped (ch30 right after ch28): Act evacuates
                    # it now, hidden behind the remaining groups' work (DVE
                    # is chain-saturated; Act's engine is idle).
                    rt_t0 = rt_pool.tile([128, 2, 256], BF16)
                    rt_t1 = rt_pool.tile([128, 2, 256], BF16)
                    nc.scalar.copy(rt_t0[:], psum_r0[:])
                    # absorber: pins the Act-copy dependency on a free DVE op
                    # so later DVE ops keep a single cross-engine wait
                    nc.vector.tensor_scalar(
                        zerod[:], rt_t0[:, 0, 0:1], 0.0, None, MULT
                    )

            # Tail: ch31's two matmuls, then DVE merges psum_x into the
            # evacuated jj0 copy while Act evacuates psum_r1; each store
            # rides a queue different from its producing engine.
            psum_x = pwx[0][:, 2:4, 0:256]
            lcp = cp_of_last[0]
            nc.tensor.matmul(
                psum_x[:], lcp[:], at_t[:, X_CH, :, 0:512],
                start=True, stop=True, perf_mode=DR,
            )
            nc.tensor.matmul(
                psum_r1[:], lcp[:], at_t[:, X_CH, :, 512:1024],
                start=False, stop=True, perf_mode=DR,
            )
            nc.vector.tensor_add(rt_t0[:], rt_t0[:], psum_x[:])
            s0 = nc.scalar.dma_start(out=rt[:, 0:512], in_=rt_t0[:])
            nc.scalar.copy(rt_t1[:], psum_r1[:])
            s1 = nc.sync.dma_start(out=rt[:, 512:1024], in_=rt_t1[:])
            store_names.update({s0.ins.name, s1.ins.name})

    # The scheduler can hoist a G1 start-Matmult ahead of the G2 Ldweights
    # whose DVE wait would dedup-cover its bank-WAR wait, leaving it with
    # two waits (PE self-wait + DVE) — one over the HW wait-slot limit.
    # The same-engine self-wait is always satisfied by in-order queue
    # completion, so drop it.
    # The rt stores' RAW wait (on the tail merge ops) transitively follows
    # every input DMA completing, so a DMA-lane credit wait on them is
    # always already satisfied — drop it to stay within the 1-wait limit.
    for inst in nc.inst_map.values():
        si = inst.sync_info
        if si and si.on_wait and len(si.on_wait) > 1:
            eng = str(inst.engine).split(".")[-1]
            is_dma = "DMA" in type(inst).__name__ or "Dma" in type(inst).__name__
            if is_dma:
                assert inst.name in store_names, (
                    inst.name,
                    [w.ant_name for w in si.on_wait],
                )
                kept = [w for w in si.on_wait if not w.ant_name.startswith("DMA")]
            else:
                kept = [w for w in si.on_wait if not w.ant_name.startswith(eng + "_")]
            assert len(kept) == 1, (inst.name, [w.ant_name for w in si.on_wait])
            si.on_wait = kept

    return nc


def _prep_inputs(address, erase, add, content):
    f8 = ml_dtypes.float8_e4m3
    a_f8 = address.astype(f8)                                 # [1024, 65536]
    ed = np.concatenate([-erase, 0.5 * add], axis=1)          # [1024, 256] f32
    ed_r = np.ascontiguousarray(
        ed.astype(f8).reshape(BCHUNKS, 128, 2 * M).transpose(1, 0, 2)
    )                                                         # [128, 8, 256]
    c_f8 = (0.5 * content).astype(f8)                         # [65536, 128]

    in_maps = []
    for ci in range(NCORES):
        a_c = a_f8[:, ci * NS : (ci + 1) * NS]                # [1024, 8192]
        # a_r[p, j, ub, bc, u] = A[bc*128+p, j*1024+ub*128+u]
        a_r = np.ascontiguousarray(
            a_c.reshape(BCHUNKS, 128, NSTAGES, 8, 128).transpose(1, 2, 3, 0, 4)
        )                                                     # [128, 8, 8, 8, 128]
        # at_r[p, ch, s, b] = A[b, ch*256 + s*128 + p]
        at_r = np.ascontiguousarray(
            a_c.T.reshape(MCHUNKS, 2, 128, B).transpose(2, 0, 1, 3)
        )                                                     # [128, 32, 2, 1024]
        c_c = c_f8[ci * NS : (ci + 1) * NS, :]
        c_r = np.ascontiguousarray(
            c_c.reshape(NCHUNKS, 128, M).transpose(1, 0, 2)
        )                                                     # [128, 64, 128]
        in_maps.append({"a": a_r, "at": at_r, "c": c_r, "ed": ed_r})
    return in_maps


def kernel(address, erase, add, content, _trace=False, _result_box=None):
    if "nc" not in _compiled:
        _compiled["nc"] = _build_nc()
    nc = _compiled["nc"]

    in_maps = _prep_inputs(address, erase, add, content)
    res = run_bass_kernel_spmd(
        nc, in_maps, core_ids=list(range(NCORES)), trace=_trace
    )
    if _result_box is not None:
        _result_box.append(res)

    acc = np.zeros((M, B), dtype=np.float32)
    for r in res.results:
        acc += np.asarray(r["rt"], dtype=np.float32)
    return np.ascontiguousarray((2.0 * acc).T)


# revision 53
# speedup vs baseline: 1.0303x; 1.0088x over previous
import sys

sys.path.insert(0, "/opt/trn_rl_repo")

import numpy as np
import ml_dtypes

import concourse.mybir as mybir
from concourse import bass, tile
from concourse import tile_sem_assignment as _tsa
from concourse.bass_utils import run_bass_kernel_spmd
from concourse.vector_clock import ScopedClock, VectorClock

_orig_drain_and_barrier = tile.TileContext._drain_and_barrier


def _split_drain_and_barrier(self, tick_clock, wait_clock):
    # The final Drain waits on every active semaphore at once; with 8 HWDGE
    # lanes + SWDGE + 3 engines that exceeds the CTRL instruction's sync
    # wait slots. Emit one 1-wait drain per proc instead (same semantics:
    # SP executes them in order, so all sems reach their targets before the
    # barrier), then replicate the original barrier/cleanup sequence.
    gc = tick_clock.global_clock
    n = _tsa.N_PROCS
    for p in range(n):
        if gc[p] > 0:
            partial = VectorClock([gc[q] if q == p else 0 for q in range(n)])
            d = self.nc.sync.drain()
            wait_clock.add_sem_waits(d.ins, ScopedClock({None: partial}))
    self.nc.all_engine_barrier()
    popped = self.nc._tile_sem_poison_stack.pop()
    assert popped is self._sem_poison
    self.nc.clear_and_free_semaphores(list(self.sems.allocated().values()))
    self.nc.all_engine_barrier()


tile.TileContext._drain_and_barrier = _split_drain_and_barrier

B = 1024        # batch rows of address
N = 65536       # mem rows (sharded)
M = 128         # mem cols
NCORES = 8
NS = N // NCORES          # 8192 rows per core
NCHUNKS = NS // 128       # 64 chunks of 128 mem-rows
MCHUNKS = NS // 256       # 32 mega-chunks of 256 mem-rows (DoubleRow)
BCHUNKS = B // 128        # 8 chunks of 128 batch-rows
NSTAGES = 8               # a-tensor j blocks (8 chunks each)

FP8 = mybir.dt.float8e4
BF16 = mybir.dt.bfloat16
F32 = mybir.dt.float32
DR = mybir.MatmulPerfMode.DoubleRow
ADD = mybir.AluOpType.add
MULT = mybir.AluOpType.mult

_compiled = {}

# G1 chunk groups: (first chunk, width, pool_pairs). The update
# (C'/2 = (1-We)(C/2) + Wa/2) is split by whole ch-pairs between DVE and
# Pool: Pool takes the last two chunks of most groups INTO ITS OWN PSUM
# TILES (psum access serializes per tile, so sharing one tile would chain
# Pool behind DVE and gate the G1 WAR on Pool's DMA-busy queue). Every G2
# weight load depends on exactly one engine's cp write (single-wait).
# Processing order: the (58,4) group runs FIRST so chs 29/30's cp is ready
# early; chunks 62,63 (ch31) run last and only their tiny chain is in the
# tail. psum_r0 accumulates [29, 0..28, 30(stop)] - the stop fires on
# long-ready data right after ch28, letting Act evacuate it mid-stream.
# psum_r1 accumulates [29, 30, 0..28, 31(stop)]. ch31's jj0 lands in psum_x
# (carved from the last group's pw slot) and merges via one DVE tensor_add.
GROUPS = (
    [(58, 4, 0)]
    + [(kk, 6, 0) for kk in range(0, 54, 6)]
    + [(54, 4, 0), (62, 2, 0)]
)

X_CH = 31


def _build_nc():
    nc = bass.Bass(target_bir_lowering=False)

    # a:  [p=b%128, j(n-slice of 1024), ub(u-block), bc, u]  A shard for GEMM1
    a = nc.dram_tensor("a", [128, NSTAGES, 8, BCHUNKS, 128], FP8, kind="ExternalInput")
    # at: [p=n%128 within 256-chunk, ch, sub, b]  A^T shard for GEMM2 (partition=n)
    at = nc.dram_tensor("at", [128, MCHUNKS, 2, B], FP8, kind="ExternalInput")
    # c:  [p=n%128, k, m]  0.5*content shard (partition=n)
    c = nc.dram_tensor("c", [128, NCHUNKS, M], FP8, kind="ExternalInput")
    # ed: [p=b%128, bc, 2M]  [-erase | 0.5*add] fp8 (hi only)
    ed = nc.dram_tensor("ed", [128, BCHUNKS, 2 * M], FP8, kind="ExternalInput")
    # rt: [m, b] partial (read/2)^T bf16
    rt = nc.dram_tensor("rt", [M, B], BF16, kind="ExternalOutput")

    store_names = set()

    with tile.TileContext(nc) as tc:
        with (
            tc.tile_pool(name="abuf", bufs=1) as a_pool,
            tc.tile_pool(name="atbuf", bufs=1) as at_pool,
            tc.tile_pool(name="cbuf", bufs=1) as c_pool,
            tc.tile_pool(name="edbuf", bufs=1) as ed_pool,
            tc.tile_pool(name="tmpbuf", bufs=6) as tmp_pool,
            tc.tile_pool(name="tmppbuf", bufs=6) as tmpp_pool,
            tc.tile_pool(name="landbuf", bufs=1) as land_pool,
            tc.tile_pool(name="landpbuf", bufs=1) as landp_pool,
            tc.tile_pool(name="wsrcbuf", bufs=1) as wsrc_pool,
            tc.tile_pool(name="cpbuf", bufs=12) as cp_pool,
            tc.tile_pool(name="rtbuf", bufs=2) as rt_pool,
            tc.tile_pool(name="warmbuf", bufs=1) as warm_pool,
            tc.tile_pool(name="pw", bufs=2, space="PSUM") as pw_pool,
            tc.tile_pool(name="pwp", bufs=2, space="PSUM") as pwp_pool,
            tc.tile_pool(name="pr", bufs=1, space="PSUM") as pr_pool,
        ):
            a_t = a_pool.tile([128, NSTAGES, 8, BCHUNKS, 128], FP8)
            at_t = at_pool.tile([128, MCHUNKS, 2, B], FP8)
            c_t = c_pool.tile([128, NCHUNKS, M], FP8)
            ed_t = ed_pool.tile([128, BCHUNKS, 2 * M], FP8)

            # Pool's first op: memset a small dummy-matmul source so PE
            # p-state warm-up can start almost immediately.
            wsrcm = wsrc_pool.tile([128, 64], FP8)
            nc.gpsimd.memset(wsrcm[:], 0.0)

            # DMA pieces, explicit queue assignment (SP/Act HWDGE, Pool
            # SWDGE), each queue's list in consumption order. First waves are
            # small so the first G1 chunks can start ~2.3us at MID clock.
            # Pool carries fewer bytes because it also runs the [MD:M] column
            # slice of every group's update.
            pieces = []
            SP, AC, PL = "sp", "ac", "pl"

            def q(which, dst, srcp):
                pieces.append((which, dst, srcp))

            # Explicit deadline-driven plan. The early phase is G1-only
            # (G2s lag the first update), demanding `a` at ~600GB/s, so `a`
            # pieces are front-loaded on all three queues in small pieces;
            # at/c slot in as their consumers approach; late at-chs park at
            # the ends. Per-queue order == consumption order.
            q(SP, ed_t[:, 0:4], ed[:, 0:4])
            q(AC, ed_t[:, 4:8], ed[:, 4:8])
            q(PL, a_t[:, 0, 0:1], a[:, 0, 0:1])
            q(SP, a_t[:, 0, 1:2], a[:, 0, 1:2])
            q(AC, a_t[:, 0, 2:3], a[:, 0, 2:3])
            q(PL, c_t[:, 0:8, :], c[:, 0:8, :])
            q(SP, a_t[:, 0, 3:4], a[:, 0, 3:4])
            q(AC, a_t[:, 0, 5:6], a[:, 0, 5:6])
            q(PL, a_t[:, 0, 4:5], a[:, 0, 4:5])
            q(SP, a_t[:, 0, 6:7], a[:, 0, 6:7])
            q(AC, a_t[:, 0, 7:8], a[:, 0, 7:8])
            q(SP, a_t[:, 1, 0:4], a[:, 1, 0:4])
            q(AC, at_t[:, 0:2], at[:, 0:2])
            q(PL, at_t[:, 2:5], at[:, 2:5])
            q(SP, a_t[:, 2, 0:4], a[:, 2, 0:4])
            q(AC, a_t[:, 1, 4:8], a[:, 1, 4:8])
            q(PL, c_t[:, 8:26, :], c[:, 8:26, :])
            q(SP, c_t[:, 26:40, :], c[:, 26:40, :])
            q(AC, at_t[:, 5:7], at[:, 5:7])
            q(PL, a_t[:, 2, 4:8], a[:, 2, 4:8])
            q(SP, a_t[:, 3, 0:4], a[:, 3, 0:4])
            q(AC, a_t[:, 3, 4:8], a[:, 3, 4:8])
            q(PL, at_t[:, 7:8], at[:, 7:8])
            q(SP, a_t[:, 4, 0:4], a[:, 4, 0:4])
            q(AC, at_t[:, 8:10], at[:, 8:10])
            q(PL, a_t[:, 5, 4:8], a[:, 5, 4:8])
            q(SP, at_t[:, 14:16], at[:, 14:16])
            q(AC, a_t[:, 4, 4:8], a[:, 4, 4:8])
            q(PL, at_t[:, 10:12], at[:, 10:12])
            q(SP, a_t[:, 5, 0:4], a[:, 5, 0:4])
            q(AC, at_t[:, 16:18], at[:, 16:18])
            q(PL, at_t[:, 12:14], at[:, 12:14])
            q(SP, a_t[:, 6, 0:4], a[:, 6, 0:4])
            q(AC, a_t[:, 6, 4:8], a[:, 6, 4:8])
            q(PL, c_t[:, 40:56, :], c[:, 40:56, :])
            q(SP, at_t[:, 20:22], at[:, 20:22])
            q(AC, at_t[:, 22:24], at[:, 22:24])
            q(PL, at_t[:, 18:20], at[:, 18:20])
            q(SP, a_t[:, 7, 0:4], a[:, 7, 0:4])
            q(AC, a_t[:, 7, 4:8], a[:, 7, 4:8])
            q(PL, at_t[:, 24:26], at[:, 24:26])
            q(SP, at_t[:, 26:28], at[:, 26:28])
            q(AC, at_t[:, 28:30], at[:, 28:30])
            q(PL, c_t[:, 56:64, :], c[:, 56:64, :])
            q(SP, at_t[:, 30:32], at[:, 30:32])

            queues = {SP: nc.sync, AC: nc.scalar, PL: nc.gpsimd}
            for which, dst, srcp in pieces:
                queues[which].dma_start(out=dst, in_=srcp)

            # Ramp the PE p-state: the model runs the Tensor engine at 1.2GHz
            # until it has been continuously busy for 3us. Small dummy matmuls
            # into psum_r0 (discarded by the first real G2's start=True) keep
            # PE busy until the first a/ed pieces land (~2.3us); the first
            # real chunks then run at MID clock until the ramp completes.
            psum_r0 = pr_pool.tile([128, 2, 256], F32)
            psum_r1 = pr_pool.tile([128, 2, 256], F32)
            psum_r = [psum_r0, psum_r1]
            for _ in range(42):
                nc.tensor.matmul(
                    psum_r0[0:1, 0, 0:64], wsrcm[:, 0:1], wsrcm[:], start=True, stop=True
                )

            # Per-partition scalar tiles for the STTs. Refreshing them with
            # ops that READ the c-DMA boundary chunks (and the recycled cp
            # slot) forces those waits onto the cheap refresher via a real
            # data edge, so the big ops keep a single cross-engine wait.
            oned = land_pool.tile([128, 1], F32)
            zerod = land_pool.tile([128, 1], F32, name="zerod")
            onep = landp_pool.tile([128, 1], F32)

            g2_first = [True]

            def emit_g2(ch, cp, out=None, start=None, stop=False):
                for jj in range(2):
                    dst = psum_r[jj][:] if out is None else out[:, jj, :]
                    nc.tensor.matmul(
                        dst,
                        cp[:],
                        at_t[:, ch, :, jj * 512 : (jj + 1) * 512],
                        start=g2_first[0] if start is None else start,
                        stop=stop,
                        perf_mode=DR,
                    )
                    if start is None:
                        g2_first[0] = False

            def g1_mms(k0, nk, psum_w, d0):
                for dk in range(nk):
                    k = k0 + dk
                    j, ub = k // 8, k % 8
                    for q4 in range(4):
                        nc.tensor.matmul(
                            psum_w[:, d0 + dk, :],
                            a_t[:, j, ub, 2 * q4 : 2 * q4 + 2, :],
                            ed_t[:, 2 * q4 : 2 * q4 + 2, :],
                            start=(q4 == 0),
                            stop=(q4 == 3),
                            perf_mode=DR,
                        )

            def emit_update(k0, w, cp, pool_pairs=0, psum_w=None, nxt=None,
                            pool_all=False):
                # psum_w[dk] = [-We | Wa/2];  C'/2 = (1 - We)*(C/2) + Wa/2
                # DVE handles chunks [0:wd] in its psum tile, Pool the last
                # pool_pairs*2 in a separate tile; the land copies let each
                # engine absorb the c-DMA wait before its STT so every
                # instruction keeps a single cross-engine wait.
                if pool_all:
                    # the final tiny group updates on Pool (its DMA queue has
                    # long drained by now), in parallel with DVE's tail
                    g1_mms(k0, w, psum_w, 0)
                    # onep := 1.0 reading the c piece, so the STT's c-wait is
                    # forced through a real data edge (scalar operand)
                    nc.gpsimd.tensor_scalar(
                        onep[:], c_t[:, k0 + w - 1, 1:2], 0.0, 1.0, MULT, ADD
                    )
                    tmp2p = tmpp_pool.tile([128, w, M], F32)
                    nc.gpsimd.scalar_tensor_tensor(
                        tmp2p[:, 0:w, :],
                        psum_w[:, 0:w, 0:M],
                        onep[:],
                        c_t[:, k0 : k0 + w, :],
                        ADD,
                        MULT,
                    )
                    nc.gpsimd.tensor_add(
                        cp[:, 0:w, :], tmp2p[:, 0:w, :], psum_w[:, 0:w, M : 2 * M]
                    )
                    return
                wd = w - 2 * pool_pairs
                if psum_w is None:
                    psum_w = pw_pool.tile([128, 6, 2 * M], F32)
                g1_mms(k0, wd, psum_w, 0)
                if pool_pairs:
                    psum_wp = pwp_pool.tile([128, 2, 2 * M], F32)
                    g1_mms(k0 + wd, 2, psum_wp, 0)
                # oned/onep are always 1.0; the refreshers exist to absorb
                # the c-DMA waits via a real data edge, and are hoisted one
                # group ahead (emitted mid-update) so their sem waits are
                # long-satisfied and add no latency to the DVE/Pool chains.
                tmp2 = tmp_pool.tile([128, wd, M], F32)
                nc.vector.scalar_tensor_tensor(
                    tmp2[:, 0:wd, :],
                    psum_w[:, 0:wd, 0:M],
                    oned[:],
                    c_t[:, k0 : k0 + wd, :],
                    ADD,
                    MULT,
                )
                for cell in (nxt or ()):
                    # span-entry refresher: pulls the new c-piece's DMA sem
                    # into DVE's wait clock one group early
                    nc.vector.tensor_scalar(
                        oned[:], c_t[:, cell, 0:1], 0.0, 1.0, MULT, ADD
                    )
                nc.vector.tensor_add(
                    cp[:, 0:wd, :], tmp2[:, 0:wd, :], psum_w[:, 0:wd, M : 2 * M]
                )
                if pool_pairs:
                    tmp2p = tmpp_pool.tile([128, 2, M], F32)
                    nc.gpsimd.scalar_tensor_tensor(
                        tmp2p[:, 0:2, :],
                        psum_wp[:, 0:2, 0:M],
                        onep[:],
                        c_t[:, k0 + wd : k0 + w, :],
                        ADD,
                        MULT,
                    )
                    if nxt is not None and nxt[1] is not None:
                        nc.gpsimd.tensor_scalar(
                            onep[:], c_t[:, nxt[1], 1:2], 0.0, 1.0, MULT, ADD
                        )
                    nc.gpsimd.tensor_add(
                        cp[:, wd:w, :], tmp2p[:, 0:2, :], psum_wp[:, 0:2, M : 2 * M]
                    )

            # The last group's psum_w and psum_x share one pw-shaped slot:
            # 8 psum banks leave no room for a third concurrent pw tile.
            pwx = [None]
            last_gi = len(GROUPS) - 1
            cp30 = [None]
            cp27 = [None]
            cp_of_last = [None]
            C_SPANS = ((0, 8), (8, 26), (26, 40), (40, 56), (56, 64))
            spans_waited = {(56, 64)}  # covered by the oned seed
            # seed the scalar tile (the hoisted refreshers keep it at 1.0)
            nc.vector.tensor_scalar(
                oned[:], c_t[:, 0, 0:1], 0.0, 1.0, MULT, ADD
            )
            for gi, (k0, w, pp) in enumerate(GROUPS):
                cp = cp_pool.tile([128, 6, M], FP8)
                # c-span cells the NEXT group newly touches (hoisted refs)
                nxt = []
                if gi + 1 <= last_gi:
                    nk0, nw, npp = GROUPS[gi + 1]
                    for k in range(nk0, nk0 + nw - 2 * npp):
                        sp = next(s for s in C_SPANS if s[0] <= k < s[1])
                        if sp not in spans_waited:
                            spans_waited.add(sp)
                            nxt.append(min(sp[1] - 1, nk0 + nw - 2 * npp - 1))
                            break
                if gi == last_gi:
                    pwx[0] = pw_pool.tile([128, 6, 2 * M], F32, name="psum_w")
                    cp_of_last[0] = cp[:, 0:2, :]
                    emit_update(k0, w, cp, pool_pairs=pp,
                                psum_w=pwx[0][:, 0:w, :], nxt=nxt)
                else:
                    emit_update(k0, w, cp, pool_pairs=pp, nxt=nxt)

                def g2mm(jj, ch, cpg, start, stop, out=None):
                    dst = out if out is not None else psum_r[jj]
                    nc.tensor.matmul(
                        dst[:],
                        cpg[:],
                        at_t[:, ch, :, jj * 512 : (jj + 1) * 512],
                        start=start,
                        stop=stop,
                        perf_mode=DR,
                    )

                for dch in range(w // 2):
                    ch = k0 // 2 + dch
                    cpg = cp[:, 2 * dch : 2 * dch + 2, :]
                    if ch == X_CH:
                        continue  # emitted in the tail
                    if ch == 29:
                        # first accumulated into both banks
                        g2mm(0, ch, cpg, True, False)
                        g2mm(1, ch, cpg, True, False)
                    elif ch == 30:
                        # jj1 early; jj0 deferred to be psum_r0's stop
                        g2mm(1, ch, cpg, False, False)
                        cp30[0] = cpg
                    elif ch == 27:
                        g2mm(0, ch, cpg, False, False)
                        cp27[0] = cpg
                    elif ch == 28:
                        # jj0 finishes first: ch28 then long-ready ch30
                        # stops the bank (feeding the critical Act copy),
                        # then jj1 catches up
                        g2mm(0, ch, cpg, False, False)
                        g2mm(0, 30, cp30[0], False, True)
                        g2mm(1, 27, cp27[0], False, False)
                        g2mm(1, ch, cpg, False, False)
                    else:
                        g2mm(0, ch, cpg, False, False)
                        g2mm(1, ch, cpg, False, False)

                if gi == 7:
                    # Warm the Activation engine's table load off the tail's
                    # critical path; the c_t read keeps it out of Act's busy
                    # early DMA window.
                    warm = warm_pool.tile([128, 1], F32)
                    nc.scalar.copy(warm[:], c_t[:, 57, 0:1])

                if gi == 10:
                    # psum_r0 stopped (ch30 right after ch28): Act evacuates
                    # it now, hidden behind the remaining groups' work (DVE
                    # is chain-saturated; Act's engine is idle).
                    rt_t0 = rt_pool.tile([128, 2, 256], BF16)
                    rt_t1 = rt_pool.tile([128, 2, 256], BF16)
                    nc.scalar.copy(rt_t0[:], psum_r0[:])

            # Tail: ch31's two matmuls, then DVE merges psum_x into the
            # evacuated jj0 copy while Act evacuates psum_r1; each store
            # rides a queue different from its producing engine.
            psum_x = pwx[0][:, 2:4, 0:256]
            lcp = cp_of_last[0]
            nc.tensor.matmul(
                psum_x[:], lcp[:], at_t[:, X_CH, :, 0:512],
                start=True, stop=True, perf_mode=DR,
            )
            nc.tensor.matmul(
                psum_r1[:], lcp[:], at_t[:, X_CH, :, 512:1024],
                start=False, stop=True, perf_mode=DR,
            )
            # absorber: pin the x-matmul (PE) dependency on a free DVE op
            # so the merge launches straight off the Act copy
            nc.vector.tensor_scalar(
                zerod[:], psum_x[:, 0, 0:1], 0.0, None, MULT
            )
            nc.vector.tensor_add(rt_t0[:], rt_t0[:], psum_x[:])
            s0 = nc.scalar.dma_start(out=rt[:, 0:512], in_=rt_t0[:])
            nc.scalar.copy(rt_t1[:], psum_r1[:])
            s1 = nc.sync.dma_start(out=rt[:, 512:1024], in_=rt_t1[:])
            store_names.update({s0.ins.name, s1.ins.name})

    # The scheduler can hoist a G1 start-Matmult ahead of the G2 Ldweights
    # whose DVE wait would dedup-cover its bank-WAR wait, leaving it with
    # two waits (PE self-wait + DVE) — one over the HW wait-slot limit.
    # The same-engine self-wait is always satisfied by in-order queue
    # completion, so drop it.
    # The rt stores' RAW wait (on the tail merge ops) transitively follows
    # every input DMA completing, so a DMA-lane credit wait on them is
    # always already satisfied — drop it to stay within the 1-wait limit.
    for inst in nc.inst_map.values():
        si = inst.sync_info
        if si and si.on_wait and len(si.on_wait) > 1:
            eng = str(inst.engine).split(".")[-1]
            is_dma = "DMA" in type(inst).__name__ or "Dma" in type(inst).__name__
            if is_dma:
                assert inst.name in store_names, (
                    inst.name,
                    [w.ant_name for w in si.on_wait],
                )
                kept = [w for w in si.on_wait if not w.ant_name.startswith("DMA")]
            else:
                kept = [w for w in si.on_wait if not w.ant_name.startswith(eng + "_")]
            assert len(kept) == 1, (inst.name, [w.ant_name for w in si.on_wait])
            si.on_wait = kept

    return nc


def _prep_inputs(address, erase, add, content):
    f8 = ml_dtypes.float8_e4m3
    a_f8 = address.astype(f8)                                 # [1024, 65536]
    ed = np.concatenate([-erase, 0.5 * add], axis=1)          # [1024, 256] f32
    ed_r = np.ascontiguousarray(
        ed.astype(f8).reshape(BCHUNKS, 128, 2 * M).transpose(1, 0, 2)
    )                                                         # [128, 8, 256]
    c_f8 = (0.5 * content).astype(f8)                         # [65536, 128]

    in_maps = []
    for ci in range(NCORES):
        a_c = a_f8[:, ci * NS : (ci + 1) * NS]                # [1024, 8192]
        # a_r[p, j, ub, bc, u] = A[bc*128+p, j*1024+ub*128+u]
        a_r = np.ascontiguousarray(
            a_c.reshape(BCHUNKS, 128, NSTAGES, 8, 128).transpose(1, 2, 3, 0, 4)
        )                                                     # [128, 8, 8, 8, 128]
        # at_r[p, ch, s, b] = A[b, ch*256 + s*128 + p]
        at_r = np.ascontiguousarray(
            a_c.T.reshape(MCHUNKS, 2, 128, B).transpose(2, 0, 1, 3)
        )                                                     # [128, 32, 2, 1024]
        c_c = c_f8[ci * NS : (ci + 1) * NS, :]
        c_r = np.ascontiguousarray(
            c_c.reshape(NCHUNKS, 128, M).transpose(1, 0, 2)
        )                                                     # [128, 64, 128]
        in_maps.append({"a": a_r, "at": at_r, "c": c_r, "ed": ed_r})
    return in_maps


def kernel(address, erase, add, content, _trace=False, _result_box=None):
    if "nc" not in _compiled:
        _compiled["nc"] = _build_nc()
    nc = _compiled["nc"]

    in_maps = _prep_inputs(address, erase, add, content)
    res = run_bass_kernel_spmd(
        nc, in_maps, core_ids=list(range(NCORES)), trace=_trace
    )
    if _result_box is not None:
        _result_box.append(res)

    acc = np.zeros((M, B), dtype=np.float32)
    for r in res.results:
        acc += np.asarray(r["rt"], dtype=np.float32)
    return np.ascontiguousarray((2.0 * acc).T)
